# revision 1
# baseline (speedup 1.0000x reference)
"""Multi-head attention (QKV proj + RoPE + softmax attention + output proj)
for Trainium2, tensor-parallel over heads across 8 NeuronCores.

Shapes (hardcoded): hidden_states [2, 2048, 2048], 16 heads x 128 head_dim.
Each core computes 2 heads end-to-end:
  q/k/v column-sharded projections -> RoPE -> scores^T -> exp (no max-sub,
  scores are ~+-7) -> column-sum via ones-matmul -> out^T = v^T @ exp^T ->
  normalize -> row-sharded O-projection partial.
Host sums the 8 partial outputs.

Device layouts:
  - X^T [2048 hidden, 4096 tokens] streamed as fp32r (full-rate PE).
  - q^T/k^T kept [128 d, tokens] per head (contraction on partitions).
  - v kept token-major [tokens, 256] (keys on partitions for out^T matmul).
  - RoPE via sign-folded permutation matmul: tmp = S @ q, then
    q_rot = q*cos + tmp*sin elementwise on DVE.
"""

import math

import numpy as np

HIDDEN = 2048
NH = 16
HD = 128
B = 2
S = 2048
T = B * S
NCORES = 8
HPC = NH // NCORES  # heads per core
CW = HPC * HD  # per-core projection width (256)
BASE = 10000.0
TB = 256  # phase-A token block
QB = 512  # phase-B query block
NKT = S // 128  # key tiles per batch (16)
NCH = HIDDEN // 128  # contraction chunks (16)

_CACHE = {}
import os as _os
VARIANT = _os.environ.get("KVAR", "gsum")


def _kernel_body(tc, aps, repeat=1, phases="ABC", variant=""):
    import concourse.bass as bass  # noqa: F401
    from concourse import mybir

    nc = tc.nc
    f32 = mybir.dt.float32
    f32r = mybir.dt.float32r
    bf16 = mybir.dt.bfloat16
    Act = mybir.ActivationFunctionType

    xt_r = aps["xt"].rearrange("(c p) t -> p c t", p=128)
    wq_r = aps["wq"].rearrange("(c p) m -> p c m", p=128)
    wk_r = aps["wk"].rearrange("(c p) m -> p c m", p=128)
    wv_r = aps["wv"].rearrange("(c p) m -> p c m", p=128)
    wo_r = aps["wo"].rearrange("(h p) n -> p h n", p=128)
    out_ap = aps["out"]

    qscale = 1.0 / math.sqrt(HD)

    with (
        tc.tile_pool(name="consts", bufs=1) as consts,
        tc.tile_pool(name="big", bufs=2) as big,
        tc.tile_pool(name="xt", bufs=2) as xtp,
        tc.tile_pool(name="rope", bufs=6) as rope,
        tc.tile_pool(name="expp", bufs=2) as expp,
        tc.tile_pool(name="small", bufs=2) as small,
        tc.tile_pool(name="stage", bufs=2) as stagep,
        tc.tile_pool(
            name="ps", bufs=(5 if variant == "ps5" else 4), space="PSUM"
        ) as psp,
        tc.tile_pool(
            name="psb", bufs=(1 if variant == "ps5" else 2), space="PSUM"
        ) as psb,
    ):
        # ---- constants ----
        wq_sb = consts.tile([128, NCH, CW], f32r, tag="wq")
        wk_sb = consts.tile([128, NCH, CW], f32r, tag="wk")
        wv_sb = consts.tile([128, NCH, CW], f32r, tag="wv")
        wo_sb = consts.tile([128, HPC, HIDDEN], bf16, tag="wo")
        cos_sb = consts.tile([128, S], bf16, tag="cos")
        sin_sb = consts.tile([128, S], bf16, tag="sin")
        st_sb = consts.tile([128, 128], bf16, tag="st")
        ones_sb = consts.tile([128, 1], bf16, tag="ones")
        bqk_sb = consts.tile([128, 4], f32, tag="bqk")
        bvb_sb = consts.tile([128, CW], bf16, tag="bvb")
        nc.sync.dma_start(out=wq_sb, in_=wq_r)
        nc.scalar.dma_start(out=wk_sb, in_=wk_r)
        nc.scalar.dma_start(out=bqk_sb, in_=aps["bqk"])
        nc.sync.dma_start(out=st_sb, in_=aps["st"])
        nc.scalar.dma_start(out=cos_sb, in_=aps["cosT"])
        nc.sync.dma_start(out=sin_sb, in_=aps["sinT"])
        nc.scalar.dma_start(out=wv_sb, in_=wv_r)
        nc.sync.dma_start(out=bvb_sb, in_=aps["bvb"])
        nc.scalar.dma_start(out=ones_sb, in_=aps["ones"])
        nc.sync.dma_start(out=wo_sb, in_=wo_r)

        def body(_=None):
            qTs, kTs, vts, oTs = {}, {}, {}, {}

            def emit_A_tb_gen(b, tbl):
                if tbl == 0:
                    qTs[b] = big.tile([128, HPC, S], bf16, tag="qT", name=f"qT{b}")
                    kTs[b] = big.tile([128, HPC, S], bf16, tag="kT", name=f"kT{b}")
                    vts[b] = big.tile([128, NKT, CW], bf16, tag="vtok", name=f"vt{b}")
                qT, kT, vtok = qTs[b], kTs[b], vts[b]
                g0 = b * S + tbl * TB
                s0 = tbl * TB
                xt_t = xtp.tile([128, NCH, TB], f32r, tag="xt")
                xeng = nc.sync if tbl % 2 == 0 else nc.scalar
                xeng.dma_start(out=xt_t, in_=xt_r[:, :, g0 : g0 + TB])
                units = []
                for h in range(HPC):
                    for qk, w_sb, bcol, scl, dstT in (
                        (0, wq_sb, h, qscale, qT),
                        (1, wk_sb, 2 + h, 1.0, kT),
                    ):
                        ps = psp.tile([128, TB], f32, tag="ps")
                        for c in range(NCH):
                            nc.tensor.matmul(
                                ps,
                                lhsT=w_sb[:, c, h * HD : (h + 1) * HD],
                                rhs=xt_t[:, c, :],
                                start=(c == 0),
                                stop=(c == NCH - 1),
                            )
                        strt = rope.tile([128, TB], bf16, tag="rt")
                        nc.scalar.activation(
                            strt, ps, Act.Identity,
                            bias=bqk_sb[:, bcol : bcol + 1], scale=scl,
                        )
                        units.append((strt, dstT, h))
                        yield
                for strt, dstT, h in units:
                    tps = psp.tile([128, TB], f32, tag="ps")
                    nc.tensor.matmul(tps, lhsT=st_sb, rhs=strt,
                                     start=True, stop=True)
                    t1 = rope.tile([128, TB], bf16, tag="rt")
                    nc.vector.tensor_mul(t1, strt, cos_sb[:, s0 : s0 + TB])
                    t2 = rope.tile([128, TB], bf16, tag="rt")
                    nc.vector.tensor_mul(t2, tps, sin_sb[:, s0 : s0 + TB])
                    nc.vector.tensor_add(dstT[:, h, s0 : s0 + TB], t1, t2)
                yield
                for sub in range(TB // 128):
                    psv = psp.tile([128, CW], f32, tag="ps")
                    for c in range(NCH):
                        nc.tensor.matmul(
                            psv,
                            lhsT=xt_t[:, c, sub * 128 : (sub + 1) * 128],
                            rhs=wv_sb[:, c, :],
                            start=(c == 0),
                            stop=(c == NCH - 1),
                        )
                    nc.vector.tensor_add(
                        vtok[:, tbl * (TB // 128) + sub, :], psv, bvb_sb
                    )
                    yield

            def emit_A_tb(b, tbl):
                for _ in emit_A_tb_gen(b, tbl):
                    pass

            def emit_B_unit(b, h, qb, feeder=None):
                if h == 0 and qb == 0:
                    oTs[b] = big.tile([128, HPC, S], bf16, tag="outT", name=f"oT{b}")
                qT, kT, vtok, outT = qTs[b], kTs[b], vts[b], oTs[b]
                q0 = qb * QB
                expT = expp.tile([128, NKT, QB], bf16, tag="expT")
                pso = psb.tile([128, QB], f32, tag="pso")
                pss = psb.tile([1, QB], f32, tag="pss")
                acc = small.tile([128, QB], f32, tag="rec", name=f"acc{b}_{h}_{qb}") if variant == "gsum" else None

                def consume(kt):
                    if variant == "gsum":
                        nc.tensor.matmul(
                            pso,
                            lhsT=vtok[:, kt, h * HD : (h + 1) * HD],
                            rhs=expT[:, kt, :],
                            start=(kt == 0),
                            stop=(kt == NKT - 1),
                        )
                        if kt == 0:
                            nc.vector.tensor_copy(acc, expT[:, 0, :])
                        else:
                            nc.vector.tensor_add(acc, acc, expT[:, kt, :])
                        return
                    if variant == "n256":
                        for hf in range(2):
                            sl = slice(hf * 256, (hf + 1) * 256)
                            nc.tensor.matmul(
                                pso[:, sl],
                                lhsT=vtok[:, kt, h * HD : (h + 1) * HD],
                                rhs=expT[:, kt, sl],
                                start=(kt == 0),
                                stop=(kt == NKT - 1),
                                skip_group_check=True,
                            )
                            nc.tensor.matmul(
                                pss[:, sl],
                                lhsT=ones_sb,
                                rhs=expT[:, kt, sl],
                                start=(kt == 0),
                                stop=(kt == NKT - 1),
                                skip_group_check=True,
                            )
                        return
                    nc.tensor.matmul(
                        pso,
                        lhsT=vtok[:, kt, h * HD : (h + 1) * HD],
                        rhs=expT[:, kt, :],
                        start=(kt == 0),
                        stop=(kt == NKT - 1),
                        skip_group_check=True,
                    )
                    nc.tensor.matmul(
                        pss,
                        lhsT=ones_sb,
                        rhs=expT[:, kt, :],
                        start=(kt == 0),
                        stop=(kt == NKT - 1),
                        skip_group_check=True,
                    )

                for kt in range(NKT):
                    ps = psp.tile([128, QB], f32, tag="ps")
                    nc.tensor.matmul(
                        ps,
                        lhsT=kT[:, h, kt * 128 : (kt + 1) * 128],
                        rhs=qT[:, h, q0 : q0 + QB],
                        start=True,
                        stop=True,
                    )
                    nc.scalar.activation(expT[:, kt, :], ps, Act.Exp)
                    if kt >= 1:
                        consume(kt - 1)
                    if feeder is not None and kt % 2 == 1:
                        next(feeder, None)
                consume(NKT - 1)
                if variant == "gsum":
                    import concourse.bass_isa as bass_isa
                    rbc = small.tile([128, QB], f32, tag="rec", name=f"rb{b}_{h}_{qb}")
                    nc.gpsimd.partition_all_reduce(
                        rbc, acc, channels=128, reduce_op=bass_isa.ReduceOp.add
                    )
                    nc.vector.reciprocal(rbc, rbc)
                    nc.vector.tensor_mul(outT[:, h, q0 : q0 + QB], pso, rbc)
                else:
                    rec = small.tile([1, QB], f32, tag="rec")
                    nc.vector.reciprocal(rec, pss)
                    rbc = small.tile([128, QB], f32, tag="rec")
                    nc.gpsimd.partition_broadcast(rbc, rec)
                    nc.vector.tensor_mul(outT[:, h, q0 : q0 + QB], pso, rbc)

            def emit_C_tt(b, tt):
                outT = oTs[b]
                r0 = b * S + tt * 128
                for half in range(2):
                    stage = stagep.tile([128, 2, QB], f32, tag="stage")
                    for sub in range(2):
                        nb = half * 2 + sub
                        psn = psp.tile([128, QB], f32, tag="ps")
                        for h in range(HPC):
                            nc.tensor.matmul(
                                psn,
                                lhsT=outT[:, h, tt * 128 : (tt + 1) * 128],
                                rhs=wo_sb[:, h, nb * QB : (nb + 1) * QB],
                                start=(h == 0),
                                stop=(h == HPC - 1),
                            )
                        nc.vector.tensor_copy(stage[:, sub, :], psn)
                    eng = nc.sync if half == 0 else nc.scalar
                    eng.dma_start(
                        out=out_ap[r0 : r0 + 128, half * 1024 : (half + 1) * 1024],
                        in_=stage.rearrange("p n q -> p (n q)"),
                    )

            NTBB = S // TB  # A blocks per batch (8)
            B_UNITS = [(h, qb) for h in range(HPC) for qb in range(S // QB)]
            if "B" not in phases:
                for b in range(B):
                    for tbl in range(NTBB):
                        emit_A_tb(b, tbl)
                    st_ = stagep.tile([128, 2, QB], f32, tag="stage")
                    nc.vector.tensor_copy(st_[:, 0, :], qTs[b][:, 0, :QB])
                    nc.sync.dma_start(
                        out=out_ap[b * S : b * S + 128, :QB], in_=st_[:, 0, :]
                    )
                return
            if variant == "feed" and "C" in phases:
                for tbl in range(NTBB):
                    emit_A_tb(0, tbl)

                def a_feed(b):
                    for tbl in range(NTBB):
                        yield from emit_A_tb_gen(b, tbl)

                fd = a_feed(1)
                for b in range(B):
                    feeder = fd if b == 0 else None
                    for qb in range(S // QB):
                        for h in range(HPC):
                            emit_B_unit(b, h, qb, feeder)
                        for i in range(QB // 128):
                            emit_C_tt(b, qb * (QB // 128) + i)
                    if b == 0:
                        for _ in fd:
                            pass
            else:
                for b in range(B):
                    for tbl in range(NTBB):
                        emit_A_tb(b, tbl)
                    if "C" in phases:
                        for qb in range(S // QB):
                            for h in range(HPC):
                                emit_B_unit(b, h, qb)
                            for i in range(QB // 128):
                                emit_C_tt(b, qb * (QB // 128) + i)
                    else:
                        for h, qb in B_UNITS:
                            emit_B_unit(b, h, qb)
            if "C" not in phases:
                for b in range(B):
                    st_ = stagep.tile([128, 2, QB], f32, tag="stage")
                    nc.vector.tensor_copy(st_[:, 0, :], oTs[b][:, 0, :QB])
                    nc.sync.dma_start(
                        out=out_ap[b * S : b * S + 128, :QB], in_=st_[:, 0, :]
                    )

        if repeat == 1:
            body()
        else:
            eng_hints = (
                mybir.EngineType.PE, mybir.EngineType.Activation,
                mybir.EngineType.DVE, mybir.EngineType.SP,
                mybir.EngineType.Pool,
            )

            def unrollable_body(iv0, unroll):
                for i in range(unroll):
                    body(iv0 + i)

            tc.For_i_unrolled_general(
                0, repeat, 1, unrollable_body, max_unroll=1,
                hint_engines=eng_hints,
            )


def _build(repeat=1, phases="ABC", variant=None):
    if variant is None:
        variant = VARIANT
    key = ("nc", repeat, phases, variant)
    if key in _CACHE:
        return _CACHE[key]
    import concourse.bacc as bacc
    import concourse.tile as tile
    from concourse import mybir

    f32 = mybir.dt.float32
    f32r = mybir.dt.float32r
    bf16 = mybir.dt.bfloat16

    nc = bacc.Bacc("TRN2", target_bir_lowering=False, debug=False)
    specs = [
        ("xt", [HIDDEN, T], f32r, "ExternalInput"),
        ("wq", [HIDDEN, CW], f32r, "ExternalInput"),
        ("wk", [HIDDEN, CW], f32r, "ExternalInput"),
        ("wv", [HIDDEN, CW], f32r, "ExternalInput"),
        ("wo", [CW, HIDDEN], bf16, "ExternalInput"),
        ("bqk", [128, 4], f32, "ExternalInput"),
        ("bvb", [128, CW], bf16, "ExternalInput"),
        ("cosT", [128, S], bf16, "ExternalInput"),
        ("sinT", [128, S], bf16, "ExternalInput"),
        ("st", [128, 128], bf16, "ExternalInput"),
        ("ones", [128, 1], bf16, "ExternalInput"),
        ("out", [T, HIDDEN], f32, "ExternalOutput"),
    ]
    aps = {}
    for name, shape, dt_, kind in specs:
        aps[name] = nc.dram_tensor(name, shape, dt_, kind=kind).ap()
    with tile.TileContext(nc) as tc:
        _kernel_body(tc, aps, repeat=repeat, phases=phases, variant=variant)
    nc.compile()
    _CACHE[key] = nc
    return nc


def _host_inputs(hidden_states, Wq, bq, Wk, bk, Wv, bv, Wo):
    import ml_dtypes

    X = np.ascontiguousarray(
        np.asarray(hidden_states, dtype=np.float32).reshape(T, HIDDEN)
    )
    XT = np.ascontiguousarray(X.T)

    inv = 1.0 / (BASE ** (np.arange(0, HD, 2, dtype=np.float32) / HD))
    t = np.arange(S, dtype=np.float32)
    freqs = np.outer(t, inv)  # [S, 64]
    emb = np.concatenate([freqs, freqs], axis=-1)  # [S, 128]
    cosT = np.ascontiguousarray(np.cos(emb).T.astype(ml_dtypes.bfloat16))  # [128, S]
    sinT = np.ascontiguousarray(np.sin(emb).T.astype(ml_dtypes.bfloat16))

    # S matrix: tmp = S_ @ q gives tmp[p] = -q[p+64] (p<64), q[p-64] (p>=64)
    # matmul computes lhsT.T @ rhs, so pass st = S_^T.
    S_ = np.zeros((128, 128), dtype=np.float32)
    for p in range(64):
        S_[p, p + 64] = -1.0
        S_[p + 64, p] = 1.0
    st = np.ascontiguousarray(S_.T.astype(ml_dtypes.bfloat16))

    ones = np.ones((128, 1), dtype=ml_dtypes.bfloat16)

    in_maps = []
    for c in range(NCORES):
        j0 = c * CW
        bq_c = np.asarray(bq[j0 : j0 + CW], dtype=np.float32)
        bk_c = np.asarray(bk[j0 : j0 + CW], dtype=np.float32)
        bv_c = np.asarray(bv[j0 : j0 + CW], dtype=np.float32)
        # ACT computes in*scale + bias, so pre-scale the q bias columns
        qs = 1.0 / math.sqrt(HD)
        bqk = np.stack(
            [bq_c[:HD] * qs, bq_c[HD:] * qs, bk_c[:HD], bk_c[HD:]], axis=1
        ).astype(np.float32)  # [128, 4]
        in_maps.append(
            {
                "xt": XT,
                "wq": np.ascontiguousarray(Wq[:, j0 : j0 + CW], dtype=np.float32),
                "wk": np.ascontiguousarray(Wk[:, j0 : j0 + CW], dtype=np.float32),
                "wv": np.ascontiguousarray(Wv[:, j0 : j0 + CW], dtype=np.float32),
                "wo": np.ascontiguousarray(np.asarray(Wo[j0 : j0 + CW, :], dtype=np.float32).astype(ml_dtypes.bfloat16)),
                "bqk": np.ascontiguousarray(bqk),
                "bvb": np.ascontiguousarray(
                    np.tile(bv_c[None, :], (128, 1)).astype(ml_dtypes.bfloat16)
                ),
                "cosT": cosT,
                "sinT": sinT,
                "st": st,
                "ones": ones,
            }
        )
    return in_maps


def kernel(hidden_states, Wq, bq, Wk, bk, Wv, bv, Wo):
    from concourse import bass_utils

    nc = _build(repeat=1)
    in_maps = _host_inputs(hidden_states, Wq, bq, Wk, bk, Wv, bv, Wo)
    res = bass_utils.run_bass_kernel_spmd(nc, in_maps, core_ids=list(range(NCORES)))
    acc = res.results[0]["out"].astype(np.float32)
    for c in range(1, NCORES):
        acc = acc + res.results[c]["out"]
    return acc.reshape(B, S, HIDDEN)



# revision 15
# speedup vs baseline: 3.3735x; 3.3735x over previous
"""Multi-head attention (QKV proj + RoPE + softmax attention + output proj)
for Trainium2, tensor-parallel over heads across 8 NeuronCores.

Shapes (hardcoded): hidden_states [2, 2048, 2048], 16 heads x 128 head_dim.
Each core computes 2 heads end-to-end:
  q/k/v column-sharded projections -> RoPE -> scores^T -> exp (no max-sub,
  scores are ~+-7) -> column-sum via ones-matmul -> out^T = v^T @ exp^T ->
  normalize -> row-sharded O-projection partial.
Host sums the 8 partial outputs.

Device layouts:
  - X^T [2048 hidden, 4096 tokens] streamed as fp32r (full-rate PE).
  - q^T/k^T kept [128 d, tokens] per head (contraction on partitions).
  - v kept token-major [tokens, 256] (keys on partitions for out^T matmul).
  - RoPE via sign-folded permutation matmul: tmp = S @ q, then
    q_rot = q*cos + tmp*sin elementwise on DVE.
"""

import math

import numpy as np

HIDDEN = 2048
NH = 16
HD = 128
B = 2
S = 2048
T = B * S
NCORES = 8
HPC = NH // NCORES  # heads per core
CW = HPC * HD  # per-core projection width (256)
BASE = 10000.0
TB = 256  # phase-A token block
QB = 512  # phase-B query block
NKT = S // 128  # key tiles per batch (16)
NCH = HIDDEN // 128  # contraction chunks (16)

_CACHE = {}
import os as _os
VARIANT = _os.environ.get("KVAR", "gsum")


def _kernel_body(tc, aps, repeat=1, phases="ABC", variant=""):
    import concourse.bass as bass  # noqa: F401
    from concourse import mybir

    nc = tc.nc
    f32 = mybir.dt.float32
    f32r = mybir.dt.float32r
    bf16 = mybir.dt.bfloat16
    Act = mybir.ActivationFunctionType

    use_gsum = variant in ("gsum", "gfeed")
    use_feed = variant in ("feed", "gfeed")

    xt_r = aps["xt"].rearrange("(c p) t -> p c t", p=128)
    wq_r = aps["wq"].rearrange("(c p) m -> p c m", p=128)
    wk_r = aps["wk"].rearrange("(c p) m -> p c m", p=128)
    wv_r = aps["wv"].rearrange("(c p) m -> p c m", p=128)
    wo_r = aps["wo"].rearrange("(h p) n -> p h n", p=128)
    out_ap = aps["out"]

    qscale = 1.0 / math.sqrt(HD)

    with (
        tc.tile_pool(name="consts", bufs=1) as consts,
        tc.tile_pool(name="big", bufs=2) as big,
        tc.tile_pool(name="xt", bufs=2) as xtp,
        tc.tile_pool(name="rope", bufs=6) as rope,
        tc.tile_pool(name="expp", bufs=2) as expp,
        tc.tile_pool(name="small", bufs=2) as small,
        tc.tile_pool(name="stage", bufs=2) as stagep,
        tc.tile_pool(
            name="ps", bufs=(5 if variant == "ps5" else 4), space="PSUM"
        ) as psp,
        tc.tile_pool(
            name="psb", bufs=(1 if variant == "ps5" else 2), space="PSUM"
        ) as psb,
    ):
        # ---- constants ----
        wq_sb = consts.tile([128, NCH, CW], f32r, tag="wq")
        wk_sb = consts.tile([128, NCH, CW], f32r, tag="wk")
        wv_sb = consts.tile([128, NCH, CW], f32r, tag="wv")
        wo_sb = consts.tile([128, HPC, HIDDEN], bf16, tag="wo")
        cos_sb = consts.tile([128, S], bf16, tag="cos")
        sin_sb = consts.tile([128, S], bf16, tag="sin")
        st_sb = consts.tile([128, 128], bf16, tag="st")
        ones_sb = consts.tile([128, 1], bf16, tag="ones")
        bqk_sb = consts.tile([128, 4], f32, tag="bqk")
        bvb_sb = consts.tile([128, CW], bf16, tag="bvb")
        nc.sync.dma_start(out=wq_sb, in_=wq_r)
        nc.scalar.dma_start(out=wk_sb, in_=wk_r)
        nc.scalar.dma_start(out=bqk_sb, in_=aps["bqk"])
        nc.sync.dma_start(out=st_sb, in_=aps["st"])
        nc.scalar.dma_start(out=cos_sb, in_=aps["cosT"])
        nc.sync.dma_start(out=sin_sb, in_=aps["sinT"])
        nc.scalar.dma_start(out=wv_sb, in_=wv_r)
        nc.sync.dma_start(out=bvb_sb, in_=aps["bvb"])
        nc.scalar.dma_start(out=ones_sb, in_=aps["ones"])
        nc.sync.dma_start(out=wo_sb, in_=wo_r)

        def body(_=None):
            qTs, kTs, vts, oTs = {}, {}, {}, {}

            def emit_A_tb_gen(b, tbl):
                if tbl == 0:
                    qTs[b] = big.tile([128, HPC, S], bf16, tag="qT", name=f"qT{b}")
                    kTs[b] = big.tile([128, HPC, S], bf16, tag="kT", name=f"kT{b}")
                    vts[b] = big.tile([128, NKT, CW], bf16, tag="vtok", name=f"vt{b}")
                qT, kT, vtok = qTs[b], kTs[b], vts[b]
                g0 = b * S + tbl * TB
                s0 = tbl * TB
                xt_t = xtp.tile([128, NCH, TB], f32r, tag="xt")
                xeng = nc.sync if tbl % 2 == 0 else nc.scalar
                xeng.dma_start(out=xt_t, in_=xt_r[:, :, g0 : g0 + TB])
                units = []
                for h in range(HPC):
                    for qk, w_sb, bcol, scl, dstT in (
                        (0, wq_sb, h, qscale, qT),
                        (1, wk_sb, 2 + h, 1.0, kT),
                    ):
                        ps = psp.tile([128, TB], f32, tag="ps")
                        for c in range(NCH):
                            nc.tensor.matmul(
                                ps,
                                lhsT=w_sb[:, c, h * HD : (h + 1) * HD],
                                rhs=xt_t[:, c, :],
                                start=(c == 0),
                                stop=(c == NCH - 1),
                            )
                        strt = rope.tile([128, TB], bf16, tag="rt")
                        nc.scalar.activation(
                            strt, ps, Act.Identity,
                            bias=bqk_sb[:, bcol : bcol + 1], scale=scl,
                        )
                        units.append((strt, dstT, h))
                        yield
                for strt, dstT, h in units:
                    tps = psp.tile([128, TB], f32, tag="ps")
                    nc.tensor.matmul(tps, lhsT=st_sb, rhs=strt,
                                     start=True, stop=True)
                    t1 = rope.tile([128, TB], bf16, tag="rt")
                    nc.vector.tensor_mul(t1, strt, cos_sb[:, s0 : s0 + TB])
                    t2 = rope.tile([128, TB], bf16, tag="rt")
                    nc.vector.tensor_mul(t2, tps, sin_sb[:, s0 : s0 + TB])
                    nc.vector.tensor_add(dstT[:, h, s0 : s0 + TB], t1, t2)
                yield
                for sub in range(TB // 128):
                    psv = psp.tile([128, CW], f32, tag="ps")
                    for c in range(NCH):
                        nc.tensor.matmul(
                            psv,
                            lhsT=xt_t[:, c, sub * 128 : (sub + 1) * 128],
                            rhs=wv_sb[:, c, :],
                            start=(c == 0),
                            stop=(c == NCH - 1),
                        )
                    nc.vector.tensor_add(
                        vtok[:, tbl * (TB // 128) + sub, :], psv, bvb_sb
                    )
                    yield

            def emit_A_tb(b, tbl):
                for _ in emit_A_tb_gen(b, tbl):
                    pass

            def emit_B_unit(b, h, qb, feeder=None):
                if h == 0 and qb == 0:
                    oTs[b] = big.tile([128, HPC, S], bf16, tag="outT", name=f"oT{b}")
                qT, kT, vtok, outT = qTs[b], kTs[b], vts[b], oTs[b]
                q0 = qb * QB
                expT = expp.tile([128, NKT, QB], bf16, tag="expT")
                pso = psb.tile([128, QB], f32, tag="pso")
                pss = None if use_gsum else psb.tile([1, QB], f32, tag="pss")
                acc = small.tile([128, QB], f32, tag="rec", name=f"acc{b}_{h}_{qb}") if use_gsum else None

                def consume(kt):
                    if use_gsum:
                        nc.tensor.matmul(
                            pso,
                            lhsT=vtok[:, kt, h * HD : (h + 1) * HD],
                            rhs=expT[:, kt, :],
                            start=(kt == 0),
                            stop=(kt == NKT - 1),
                        )
                        if kt == 0:
                            nc.vector.tensor_copy(acc, expT[:, 0, :])
                        else:
                            nc.vector.tensor_add(acc, acc, expT[:, kt, :])
                        return
                    if variant == "n256":
                        for hf in range(2):
                            sl = slice(hf * 256, (hf + 1) * 256)
                            nc.tensor.matmul(
                                pso[:, sl],
                                lhsT=vtok[:, kt, h * HD : (h + 1) * HD],
                                rhs=expT[:, kt, sl],
                                start=(kt == 0),
                                stop=(kt == NKT - 1),
                                skip_group_check=True,
                            )
                            nc.tensor.matmul(
                                pss[:, sl],
                                lhsT=ones_sb,
                                rhs=expT[:, kt, sl],
                                start=(kt == 0),
                                stop=(kt == NKT - 1),
                                skip_group_check=True,
                            )
                        return
                    nc.tensor.matmul(
                        pso,
                        lhsT=vtok[:, kt, h * HD : (h + 1) * HD],
                        rhs=expT[:, kt, :],
                        start=(kt == 0),
                        stop=(kt == NKT - 1),
                        skip_group_check=True,
                    )
                    nc.tensor.matmul(
                        pss,
                        lhsT=ones_sb,
                        rhs=expT[:, kt, :],
                        start=(kt == 0),
                        stop=(kt == NKT - 1),
                        skip_group_check=True,
                    )

                for kt in range(NKT):
                    ps = psp.tile([128, QB], f32, tag="ps")
                    nc.tensor.matmul(
                        ps,
                        lhsT=kT[:, h, kt * 128 : (kt + 1) * 128],
                        rhs=qT[:, h, q0 : q0 + QB],
                        start=True,
                        stop=True,
                    )
                    nc.scalar.activation(expT[:, kt, :], ps, Act.Exp)
                    if kt >= 1:
                        consume(kt - 1)
                    if feeder is not None and kt % 2 == 1:
                        next(feeder, None)
                consume(NKT - 1)
                if use_gsum:
                    import concourse.bass_isa as bass_isa
                    rbc = small.tile([128, QB], f32, tag="rec", name=f"rb{b}_{h}_{qb}")
                    nc.gpsimd.partition_all_reduce(
                        rbc, acc, channels=128, reduce_op=bass_isa.ReduceOp.add
                    )
                    nc.vector.reciprocal(rbc, rbc)
                    nc.vector.tensor_mul(outT[:, h, q0 : q0 + QB], pso, rbc)
                else:
                    rec = small.tile([1, QB], f32, tag="rec")
                    nc.vector.reciprocal(rec, pss)
                    rbc = small.tile([128, QB], f32, tag="rec")
                    nc.gpsimd.partition_broadcast(rbc, rec)
                    nc.vector.tensor_mul(outT[:, h, q0 : q0 + QB], pso, rbc)

            def emit_C_tt(b, tt):
                outT = oTs[b]
                r0 = b * S + tt * 128
                for half in range(2):
                    stage = stagep.tile([128, 2, QB], f32, tag="stage")
                    for sub in range(2):
                        nb = half * 2 + sub
                        psn = psp.tile([128, QB], f32, tag="ps")
                        for h in range(HPC):
                            nc.tensor.matmul(
                                psn,
                                lhsT=outT[:, h, tt * 128 : (tt + 1) * 128],
                                rhs=wo_sb[:, h, nb * QB : (nb + 1) * QB],
                                start=(h == 0),
                                stop=(h == HPC - 1),
                            )
                        nc.vector.tensor_copy(stage[:, sub, :], psn)
                    eng = nc.sync if half == 0 else nc.scalar
                    eng.dma_start(
                        out=out_ap[r0 : r0 + 128, half * 1024 : (half + 1) * 1024],
                        in_=stage.rearrange("p n q -> p (n q)"),
                    )

            NTBB = S // TB  # A blocks per batch (8)
            B_UNITS = [(h, qb) for h in range(HPC) for qb in range(S // QB)]
            if "B" not in phases:
                for b in range(B):
                    for tbl in range(NTBB):
                        emit_A_tb(b, tbl)
                    st_ = stagep.tile([128, 2, QB], f32, tag="stage")
                    nc.vector.tensor_copy(st_[:, 0, :], qTs[b][:, 0, :QB])
                    nc.sync.dma_start(
                        out=out_ap[b * S : b * S + 128, :QB], in_=st_[:, 0, :]
                    )
                return
            if use_feed and "C" in phases:
                for tbl in range(NTBB):
                    emit_A_tb(0, tbl)

                def a_feed(b):
                    for tbl in range(NTBB):
                        yield from emit_A_tb_gen(b, tbl)

                fd = a_feed(1)
                for b in range(B):
                    feeder = fd if b == 0 else None
                    for qb in range(S // QB):
                        for h in range(HPC):
                            emit_B_unit(b, h, qb, feeder)
                        for i in range(QB // 128):
                            emit_C_tt(b, qb * (QB // 128) + i)
                    if b == 0:
                        for _ in fd:
                            pass
            else:
                for b in range(B):
                    for tbl in range(NTBB):
                        emit_A_tb(b, tbl)
                    if "C" in phases:
                        for qb in range(S // QB):
                            for h in range(HPC):
                                emit_B_unit(b, h, qb)
                            for i in range(QB // 128):
                                emit_C_tt(b, qb * (QB // 128) + i)
                    else:
                        for h, qb in B_UNITS:
                            emit_B_unit(b, h, qb)
            if "C" not in phases:
                for b in range(B):
                    st_ = stagep.tile([128, 2, QB], f32, tag="stage")
                    nc.vector.tensor_copy(st_[:, 0, :], oTs[b][:, 0, :QB])
                    nc.sync.dma_start(
                        out=out_ap[b * S : b * S + 128, :QB], in_=st_[:, 0, :]
                    )

        if repeat == 1:
            body()
        else:
            eng_hints = (
                mybir.EngineType.PE, mybir.EngineType.Activation,
                mybir.EngineType.DVE, mybir.EngineType.SP,
                mybir.EngineType.Pool,
            )

            def unrollable_body(iv0, unroll):
                for i in range(unroll):
                    body(iv0 + i)

            tc.For_i_unrolled_general(
                0, repeat, 1, unrollable_body, max_unroll=1,
                hint_engines=eng_hints,
            )


def _kernel_body_v2(tc, aps, repeat=1, variant="v2"):
    """bf16 A-phase, in-place bf16 softmax-denominator tree, pipelined C.

    Schedule per batch: B(h0,qb) | C(qb-1) first half | B(h1,qb) | C(qb-1)
    second half; A(b1) units fed into b0's B-unit kt loops (gfeed).
    """
    import concourse.bass as bass  # noqa: F401
    import concourse.bass_isa as bass_isa
    from concourse import mybir

    nc = tc.nc
    f32 = mybir.dt.float32
    bf16 = mybir.dt.bfloat16
    Act = mybir.ActivationFunctionType

    xt_r = aps["xt"].rearrange("(c p) t -> p c t", p=128)
    wq_r = aps["wq"].rearrange("(c p) m -> p c m", p=128)
    wk_r = aps["wk"].rearrange("(c p) m -> p c m", p=128)
    wv_r = aps["wv"].rearrange("(c p) m -> p c m", p=128)
    wo_r = aps["wo"].rearrange("(h p) n -> p h n", p=128)
    out_ap = aps["out"]

    qscale = 1.0 / math.sqrt(HD)
    NTBB = S // TB  # A blocks per batch (8)
    NQB = S // QB  # B query blocks per batch (4)
    NTT = QB // 128  # C token tiles per query block (4)
    NNB = HIDDEN // QB  # C column chunks per token tile (4)

    with (
        tc.tile_pool(name="consts", bufs=1) as consts,
        tc.tile_pool(name="big", bufs=2) as big,
        tc.tile_pool(name="xt", bufs=2) as xtp,
        tc.tile_pool(name="rope", bufs=6) as rope,
        tc.tile_pool(name="expp", bufs=2) as expp,
        tc.tile_pool(name="small", bufs=2) as small,
        tc.tile_pool(name="stage", bufs=4) as stagep,
        tc.tile_pool(name="ps", bufs=4, space="PSUM") as psp,
        tc.tile_pool(name="psb", bufs=2, space="PSUM") as psb,
        tc.tile_pool(name="psc", bufs=2, space="PSUM") as psc,
    ):
        # ---- constants ----
        wq_sb = consts.tile([128, NCH, CW], bf16, tag="wq")
        wk_sb = consts.tile([128, NCH, CW], bf16, tag="wk")
        wv_sb = consts.tile([128, NCH, CW], bf16, tag="wv")
        wo_sb = consts.tile([128, HPC, HIDDEN], bf16, tag="wo")
        cos_sb = consts.tile([128, S], bf16, tag="cos")
        sin_sb = consts.tile([128, S], bf16, tag="sin")
        st_sb = consts.tile([128, 128], bf16, tag="st")
        bqk_sb = consts.tile([128, 4], f32, tag="bqk")
        bvb_sb = consts.tile([128, CW], bf16, tag="bvb")
        nc.sync.dma_start(out=wq_sb, in_=wq_r)
        nc.scalar.dma_start(out=wk_sb, in_=wk_r)
        nc.scalar.dma_start(out=bqk_sb, in_=aps["bqk"])
        nc.sync.dma_start(out=st_sb, in_=aps["st"])
        nc.scalar.dma_start(out=cos_sb, in_=aps["cosT"])
        nc.sync.dma_start(out=sin_sb, in_=aps["sinT"])
        nc.scalar.dma_start(out=wv_sb, in_=wv_r)
        nc.sync.dma_start(out=bvb_sb, in_=aps["bvb"])
        nc.sync.dma_start(out=wo_sb, in_=wo_r)

        def body(_=None):
            qTs, kTs, vts, oTs = {}, {}, {}, {}

            def emit_A_tb_gen(b, tbl):
                if tbl == 0:
                    qTs[b] = big.tile([128, HPC, S], bf16, tag="qT", name=f"qT{b}")
                    kTs[b] = big.tile([128, HPC, S], bf16, tag="kT", name=f"kT{b}")
                    vts[b] = big.tile([128, NKT, CW], bf16, tag="vtok", name=f"vt{b}")
                qT, kT, vtok = qTs[b], kTs[b], vts[b]
                g0 = b * S + tbl * TB
                s0 = tbl * TB
                xt_t = xtp.tile([128, NCH, TB], bf16, tag="xt")
                xeng = nc.sync if tbl % 2 == 0 else nc.scalar
                xeng.dma_start(out=xt_t, in_=xt_r[:, :, g0 : g0 + TB])
                units = []
                for h in range(HPC):
                    for qk, w_sb, bcol, scl, dstT in (
                        (0, wq_sb, h, qscale, qT),
                        (1, wk_sb, 2 + h, 1.0, kT),
                    ):
                        ps = psp.tile([128, TB], f32, tag="ps")
                        for c in range(NCH):
                            nc.tensor.matmul(
                                ps,
                                lhsT=w_sb[:, c, h * HD : (h + 1) * HD],
                                rhs=xt_t[:, c, :],
                                start=(c == 0),
                                stop=(c == NCH - 1),
                            )
                        strt = rope.tile([128, TB], bf16, tag="rt")
                        nc.scalar.activation(
                            strt, ps, Act.Identity,
                            bias=bqk_sb[:, bcol : bcol + 1], scale=scl,
                        )
                        units.append((strt, dstT, h))
                        yield
                for strt, dstT, h in units:
                    tps = psp.tile([128, TB], f32, tag="ps")
                    nc.tensor.matmul(tps, lhsT=st_sb, rhs=strt,
                                     start=True, stop=True)
                    t1 = rope.tile([128, TB], bf16, tag="rt")
                    nc.vector.tensor_mul(t1, strt, cos_sb[:, s0 : s0 + TB])
                    t2 = rope.tile([128, TB], bf16, tag="rt")
                    nc.vector.tensor_mul(t2, tps, sin_sb[:, s0 : s0 + TB])
                    nc.vector.tensor_add(dstT[:, h, s0 : s0 + TB], t1, t2)
                yield
                for sub in range(TB // 128):
                    psv = psp.tile([128, CW], f32, tag="ps")
                    for c in range(NCH):
                        nc.tensor.matmul(
                            psv,
                            lhsT=xt_t[:, c, sub * 128 : (sub + 1) * 128],
                            rhs=wv_sb[:, c, :],
                            start=(c == 0),
                            stop=(c == NCH - 1),
                        )
                    nc.vector.tensor_add(
                        vtok[:, tbl * (TB // 128) + sub, :], psv, bvb_sb
                    )
                    yield

            def emit_A_tb(b, tbl):
                for _ in emit_A_tb_gen(b, tbl):
                    pass

            from collections import deque
            from functools import partial

            qa = deque()  # A-phase feed steps (PE-heavy, ~1.7us each)
            qc = deque()  # C-phase chunks (PE-light, ~0.9us each)

            def pump(kt):
                if kt % 2 == 1:
                    if qa:
                        qa.popleft()()
                    elif qc:
                        qc.popleft()()
                elif qc:
                    qc.popleft()()

            def emit_B_unit(b, h, qb):
                if h == 0 and qb == 0:
                    oTs[b] = big.tile([128, HPC, S], bf16, tag="outT", name=f"oT{b}")
                qT, kT, vtok, outT = qTs[b], kTs[b], vts[b], oTs[b]
                q0 = qb * QB
                expT = expp.tile([128, NKT, QB], bf16, tag="expT")
                pso = psb.tile([128, QB], f32, tag="pso")

                def consume(kt):
                    nc.tensor.matmul(
                        pso,
                        lhsT=vtok[:, kt, h * HD : (h + 1) * HD],
                        rhs=expT[:, kt, :],
                        start=(kt == 0),
                        stop=(kt == NKT - 1),
                    )
                    if kt % 2 == 1:
                        # bf16 in-place pair sum for the softmax denominator
                        nc.vector.tensor_add(
                            expT[:, kt - 1, :], expT[:, kt - 1, :],
                            expT[:, kt, :],
                        )

                for kt in range(NKT):
                    ps = psp.tile([128, QB], f32, tag="ps")
                    nc.tensor.matmul(
                        ps,
                        lhsT=kT[:, h, kt * 128 : (kt + 1) * 128],
                        rhs=qT[:, h, q0 : q0 + QB],
                        start=True,
                        stop=True,
                    )
                    nc.scalar.activation(expT[:, kt, :], ps, Act.Exp)
                    if kt >= 1:
                        consume(kt - 1)
                    pump(kt)
                consume(NKT - 1)
                for stp in (4, 8, 16):
                    for j0 in range(0, NKT, stp):
                        nc.vector.tensor_add(
                            expT[:, j0, :], expT[:, j0, :],
                            expT[:, j0 + stp // 2, :],
                        )
                sf = small.tile([128, QB], f32, tag="rec", name=f"sf{b}_{h}_{qb}")
                nc.vector.tensor_copy(sf, expT[:, 0, :])
                rbc = small.tile([128, QB], f32, tag="rec", name=f"rb{b}_{h}_{qb}")
                nc.gpsimd.partition_all_reduce(
                    rbc, sf, channels=128, reduce_op=bass_isa.ReduceOp.add
                )
                nc.vector.reciprocal(rbc, rbc)
                nc.vector.tensor_mul(outT[:, h, q0 : q0 + QB], pso, rbc)

            ccnt = [0]

            def emit_C_chunk(b, tt, nb):
                outT = oTs[b]
                r0 = b * S + tt * 128
                psn = psc.tile([128, QB], f32, tag="psn")
                for h in range(HPC):
                    nc.tensor.matmul(
                        psn,
                        lhsT=outT[:, h, tt * 128 : (tt + 1) * 128],
                        rhs=wo_sb[:, h, nb * QB : (nb + 1) * QB],
                        start=(h == 0),
                        stop=(h == HPC - 1),
                    )
                stage = stagep.tile([128, QB], f32, tag="stage")
                i = ccnt[0]
                ccnt[0] += 1
                if i % 2 == 0:
                    nc.vector.tensor_copy(stage, psn)
                else:
                    nc.scalar.activation(stage, psn, Act.Identity)
                q = nc.sync if i % 2 == 0 else nc.scalar
                q.dma_start(
                    out=out_ap[r0 : r0 + 128, nb * QB : (nb + 1) * QB],
                    in_=stage,
                )

            def push_C(b, qb):
                for i in range(NTT):
                    for nb in range(NNB):
                        qc.append(partial(emit_C_chunk, b, qb * NTT + i, nb))

            for tbl in range(NTBB):
                emit_A_tb(0, tbl)

            _DONE = object()
            if B > 1:
                a_gen = (
                    step for tbl in range(NTBB)
                    for step in emit_A_tb_gen(1, tbl)
                )

                def a_step():
                    if next(a_gen, _DONE) is not _DONE:
                        qa.append(a_step)

                qa.append(a_step)

            for b in range(B):
                for qb in range(NQB):
                    emit_B_unit(b, 0, qb)
                    emit_B_unit(b, 1, qb)
                    push_C(b, qb)
                if b == 0:
                    # A(b1) must fully land before B(b1) starts; C chunks
                    # still queued ride into b1's pumps.
                    while qa:
                        qa.popleft()()
                        if qc:
                            qc.popleft()()
            while qa or qc:
                if qa:
                    qa.popleft()()
                if qc:
                    qc.popleft()()

        if repeat == 1:
            body()
        else:
            eng_hints = (
                mybir.EngineType.PE, mybir.EngineType.Activation,
                mybir.EngineType.DVE, mybir.EngineType.SP,
                mybir.EngineType.Pool,
            )

            def unrollable_body(iv0, unroll):
                for i in range(unroll):
                    body(iv0 + i)

            tc.For_i_unrolled_general(
                0, repeat, 1, unrollable_body, max_unroll=1,
                hint_engines=eng_hints,
            )


def _kernel_body_v3(tc, aps, repeat=1, variant="v3"):
    """v2 + 2-bank PSUM pairs: one Exp per two score tiles, paired C
    chunks with single [128,1024] copy+DMA, all PSUM from a shared
    3-deep pair ring (6 banks) + double-buffered pso (2 banks).
    """
    import concourse.bass as bass  # noqa: F401
    import concourse.bass_isa as bass_isa
    from concourse import mybir

    nc = tc.nc
    f32 = mybir.dt.float32
    bf16 = mybir.dt.bfloat16
    Act = mybir.ActivationFunctionType

    xt_r = aps["xt"].rearrange("(c p) t -> p c t", p=128)
    wq_r = aps["wq"].rearrange("(c p) m -> p c m", p=128)
    wk_r = aps["wk"].rearrange("(c p) m -> p c m", p=128)
    wv_r = aps["wv"].rearrange("(c p) m -> p c m", p=128)
    wo_r = aps["wo"].rearrange("(h p) n -> p h n", p=128)
    out_ap = aps["out"]

    qscale = 1.0 / math.sqrt(HD)
    NTBB = S // TB
    NQB = S // QB
    NTT = QB // 128
    NNB = HIDDEN // QB

    with (
        tc.tile_pool(name="consts", bufs=1) as consts,
        tc.tile_pool(name="big", bufs=2) as big,
        tc.tile_pool(name="xt", bufs=2) as xtp,
        tc.tile_pool(name="rope", bufs=6) as rope,
        tc.tile_pool(name="expp", bufs=2) as expp,
        tc.tile_pool(name="small", bufs=2) as small,
        tc.tile_pool(name="stage", bufs=4) as stagep,
        tc.tile_pool(name="pair", bufs=3, space="PSUM") as pairp,
        tc.tile_pool(name="psb", bufs=2, space="PSUM") as psb,
    ):
        # ---- constants ----
        wq_sb = consts.tile([128, NCH, CW], bf16, tag="wq")
        wk_sb = consts.tile([128, NCH, CW], bf16, tag="wk")
        wv_sb = consts.tile([128, NCH, CW], bf16, tag="wv")
        wo_sb = consts.tile([128, HPC, HIDDEN], bf16, tag="wo")
        cos_sb = consts.tile([128, S], bf16, tag="cos")
        sin_sb = consts.tile([128, S], bf16, tag="sin")
        st_sb = consts.tile([128, 128], bf16, tag="st")
        bqk_sb = consts.tile([128, 4], f32, tag="bqk")
        bvb_sb = consts.tile([128, CW], bf16, tag="bvb")
        nc.sync.dma_start(out=wq_sb, in_=wq_r)
        nc.scalar.dma_start(out=wk_sb, in_=wk_r)
        nc.scalar.dma_start(out=bqk_sb, in_=aps["bqk"])
        nc.sync.dma_start(out=st_sb, in_=aps["st"])
        nc.scalar.dma_start(out=cos_sb, in_=aps["cosT"])
        nc.sync.dma_start(out=sin_sb, in_=aps["sinT"])
        nc.scalar.dma_start(out=wv_sb, in_=wv_r)
        nc.sync.dma_start(out=bvb_sb, in_=aps["bvb"])
        nc.sync.dma_start(out=wo_sb, in_=wo_r)

        def body(_=None):
            qTs, kTs, vts, oTs = {}, {}, {}, {}

            def emit_A_tb_gen(b, tbl):
                if tbl == 0:
                    qTs[b] = big.tile([128, HPC, S], bf16, tag="qT", name=f"qT{b}")
                    kTs[b] = big.tile([128, HPC, S], bf16, tag="kT", name=f"kT{b}")
                    vts[b] = big.tile([128, NKT, CW], bf16, tag="vtok", name=f"vt{b}")
                qT, kT, vtok = qTs[b], kTs[b], vts[b]
                g0 = b * S + tbl * TB
                s0 = tbl * TB
                xt_t = xtp.tile([128, NCH, TB], bf16, tag="xt")
                xeng = nc.sync if tbl % 2 == 0 else nc.scalar
                xeng.dma_start(out=xt_t, in_=xt_r[:, :, g0 : g0 + TB])
                units = []
                for h in range(HPC):
                    pst = pairp.tile([128, 2, TB], f32, tag="pair")
                    for qk, w_sb, bcol, scl, dstT in (
                        (0, wq_sb, h, qscale, qT),
                        (1, wk_sb, 2 + h, 1.0, kT),
                    ):
                        for c in range(NCH):
                            nc.tensor.matmul(
                                pst[:, qk, :],
                                lhsT=w_sb[:, c, h * HD : (h + 1) * HD],
                                rhs=xt_t[:, c, :],
                                start=(c == 0),
                                stop=(c == NCH - 1),
                            )
                        strt = rope.tile([128, TB], bf16, tag="rt")
                        nc.scalar.activation(
                            strt, pst[:, qk, :], Act.Identity,
                            bias=bqk_sb[:, bcol : bcol + 1], scale=scl,
                        )
                        units.append((strt, dstT, h))
                        yield
                for ui in range(0, len(units), 2):
                    psr = pairp.tile([128, 2, TB], f32, tag="pair")
                    for sl in range(2):
                        strt, dstT, h = units[ui + sl]
                        nc.tensor.matmul(psr[:, sl, :], lhsT=st_sb, rhs=strt,
                                         start=True, stop=True)
                        t1 = rope.tile([128, TB], bf16, tag="rt")
                        nc.vector.tensor_mul(t1, strt, cos_sb[:, s0 : s0 + TB])
                        t2 = rope.tile([128, TB], bf16, tag="rt")
                        nc.vector.tensor_mul(t2, psr[:, sl, :], sin_sb[:, s0 : s0 + TB])
                        nc.vector.tensor_add(dstT[:, h, s0 : s0 + TB], t1, t2)
                yield
                psv = pairp.tile([128, 2, CW], f32, tag="pair")
                for sub in range(TB // 128):
                    for c in range(NCH):
                        nc.tensor.matmul(
                            psv[:, sub, :],
                            lhsT=xt_t[:, c, sub * 128 : (sub + 1) * 128],
                            rhs=wv_sb[:, c, :],
                            start=(c == 0),
                            stop=(c == NCH - 1),
                        )
                    nc.vector.tensor_add(
                        vtok[:, tbl * (TB // 128) + sub, :], psv[:, sub, :],
                        bvb_sb,
                    )
                    yield

            def emit_A_tb(b, tbl):
                for _ in emit_A_tb_gen(b, tbl):
                    pass

            from collections import deque
            from functools import partial

            qa = deque()  # A-phase feed steps (PE-heavy)
            qc = deque()  # C-phase chunks

            def pump(kt):
                if kt % 2 == 1:
                    if qa:
                        qa.popleft()()
                    elif qc:
                        qc.popleft()()
                elif qc:
                    qc.popleft()()

            def emit_B_unit(b, h, qb):
                if h == 0 and qb == 0:
                    oTs[b] = big.tile([128, HPC, S], bf16, tag="outT", name=f"oT{b}")
                qT, kT, vtok, outT = qTs[b], kTs[b], vts[b], oTs[b]
                q0 = qb * QB
                expT = expp.tile([128, NKT, QB], bf16, tag="expT")
                pso = psb.tile([128, QB], f32, tag="pso")

                def consume(kt):
                    nc.tensor.matmul(
                        pso,
                        lhsT=vtok[:, kt, h * HD : (h + 1) * HD],
                        rhs=expT[:, kt, :],
                        start=(kt == 0),
                        stop=(kt == NKT - 1),
                    )
                    if kt % 2 == 1:
                        # bf16 in-place pair sum for the softmax denominator
                        nc.vector.tensor_add(
                            expT[:, kt - 1, :], expT[:, kt - 1, :],
                            expT[:, kt, :],
                        )

                for kp in range(NKT // 2):
                    pp = pairp.tile([128, 2, QB], f32, tag="pair")
                    for sl in range(2):
                        nc.tensor.matmul(
                            pp[:, sl, :],
                            lhsT=kT[:, h, (2 * kp + sl) * 128 : (2 * kp + sl + 1) * 128],
                            rhs=qT[:, h, q0 : q0 + QB],
                            start=True,
                            stop=True,
                        )
                    nc.scalar.activation(
                        expT[:, 2 * kp : 2 * kp + 2, :], pp, Act.Exp
                    )
                    if kp >= 1:
                        consume(2 * kp - 2)
                        consume(2 * kp - 1)
                    pump(2 * kp)
                    pump(2 * kp + 1)
                consume(NKT - 2)
                consume(NKT - 1)
                for stp in (4, 8, 16):
                    for j0 in range(0, NKT, stp):
                        nc.vector.tensor_add(
                            expT[:, j0, :], expT[:, j0, :],
                            expT[:, j0 + stp // 2, :],
                        )
                sf = small.tile([128, QB], f32, tag="rec", name=f"sf{b}_{h}_{qb}")
                nc.vector.tensor_copy(sf, expT[:, 0, :])
                rbc = small.tile([128, QB], f32, tag="rec", name=f"rb{b}_{h}_{qb}")
                nc.gpsimd.partition_all_reduce(
                    rbc, sf, channels=128, reduce_op=bass_isa.ReduceOp.add
                )
                nc.vector.reciprocal(rbc, rbc)
                nc.vector.tensor_mul(outT[:, h, q0 : q0 + QB], pso, rbc)

            ccnt = [0]

            def emit_C_pair(b, tt, nbp):
                outT = oTs[b]
                r0 = b * S + tt * 128
                psn = pairp.tile([128, 2, QB], f32, tag="pair")
                for sl in range(2):
                    for h in range(HPC):
                        nc.tensor.matmul(
                            psn[:, sl, :],
                            lhsT=outT[:, h, tt * 128 : (tt + 1) * 128],
                            rhs=wo_sb[:, h, (2 * nbp + sl) * QB : (2 * nbp + sl + 1) * QB],
                            start=(h == 0),
                            stop=(h == HPC - 1),
                        )
                stage = stagep.tile([128, 2, QB], f32, tag="stage")
                i = ccnt[0]
                ccnt[0] += 1
                if i % 4 == 3:
                    nc.scalar.activation(stage, psn, Act.Identity)
                else:
                    nc.vector.tensor_copy(stage, psn)
                q = nc.sync if i % 2 == 0 else nc.scalar
                q.dma_start(
                    out=out_ap[r0 : r0 + 128, 2 * nbp * QB : 2 * (nbp + 1) * QB],
                    in_=stage.rearrange("p n q -> p (n q)"),
                )

            def push_C(b, qb):
                for i in range(NTT):
                    for nbp in range(NNB // 2):
                        qc.append(partial(emit_C_pair, b, qb * NTT + i, nbp))

            for tbl in range(NTBB):
                emit_A_tb(0, tbl)

            _DONE = object()
            if B > 1:
                a_gen = (
                    step for tbl in range(NTBB)
                    for step in emit_A_tb_gen(1, tbl)
                )

                def a_step():
                    if next(a_gen, _DONE) is not _DONE:
                        qa.append(a_step)

                qa.append(a_step)

            for b in range(B):
                for qb in range(NQB):
                    emit_B_unit(b, 0, qb)
                    emit_B_unit(b, 1, qb)
                    push_C(b, qb)
                if b == 0:
                    while qa:
                        qa.popleft()()
                        if qc:
                            qc.popleft()()
            while qa or qc:
                if qa:
                    qa.popleft()()
                if qc:
                    qc.popleft()()

        if repeat == 1:
            body()
        else:
            eng_hints = (
                mybir.EngineType.PE, mybir.EngineType.Activation,
                mybir.EngineType.DVE, mybir.EngineType.SP,
                mybir.EngineType.Pool,
            )

            def unrollable_body(iv0, unroll):
                for i in range(unroll):
                    body(iv0 + i)

            tc.For_i_unrolled_general(
                0, repeat, 1, unrollable_body, max_unroll=1,
                hint_engines=eng_hints,
            )


def _build(repeat=1, phases="ABC", variant=None):
    if variant is None:
        variant = VARIANT
    key = ("nc", repeat, phases, variant)
    if key in _CACHE:
        return _CACHE[key]
    import concourse.bacc as bacc
    import concourse.tile as tile
    from concourse import mybir

    f32 = mybir.dt.float32
    f32r = mybir.dt.float32r
    bf16 = mybir.dt.bfloat16

    nc = bacc.Bacc("TRN2", target_bir_lowering=False, debug=False)
    v2 = variant.startswith("v2")
    v3 = variant.startswith("v3")
    in_dt = bf16 if (v2 or v3) else f32r
    specs = [
        ("xt", [HIDDEN, T], in_dt, "ExternalInput"),
        ("wq", [HIDDEN, CW], in_dt, "ExternalInput"),
        ("wk", [HIDDEN, CW], in_dt, "ExternalInput"),
        ("wv", [HIDDEN, CW], in_dt, "ExternalInput"),
        ("wo", [CW, HIDDEN], bf16, "ExternalInput"),
        ("bqk", [128, 4], f32, "ExternalInput"),
        ("bvb", [128, CW], bf16, "ExternalInput"),
        ("cosT", [128, S], bf16, "ExternalInput"),
        ("sinT", [128, S], bf16, "ExternalInput"),
        ("st", [128, 128], bf16, "ExternalInput"),
        ("ones", [128, 1], bf16, "ExternalInput"),
        ("out", [T, HIDDEN], f32, "ExternalOutput"),
    ]
    aps = {}
    for name, shape, dt_, kind in specs:
        aps[name] = nc.dram_tensor(name, shape, dt_, kind=kind).ap()
    with tile.TileContext(nc) as tc:
        if v3:
            _kernel_body_v3(tc, aps, repeat=repeat, variant=variant)
        elif v2:
            _kernel_body_v2(tc, aps, repeat=repeat, variant=variant)
        else:
            _kernel_body(tc, aps, repeat=repeat, phases=phases, variant=variant)
    nc.compile()
    _CACHE[key] = nc
    return nc


def _host_inputs(hidden_states, Wq, bq, Wk, bk, Wv, bv, Wo, variant=None):
    import ml_dtypes

    if variant is None:
        variant = VARIANT
    v2 = variant.startswith("v2") or variant.startswith("v3")
    in_np = ml_dtypes.bfloat16 if v2 else np.float32

    X = np.ascontiguousarray(
        np.asarray(hidden_states, dtype=np.float32).reshape(T, HIDDEN)
    )
    XT = np.ascontiguousarray(X.T.astype(in_np))

    inv = 1.0 / (BASE ** (np.arange(0, HD, 2, dtype=np.float32) / HD))
    t = np.arange(S, dtype=np.float32)
    freqs = np.outer(t, inv)  # [S, 64]
    emb = np.concatenate([freqs, freqs], axis=-1)  # [S, 128]
    cosT = np.ascontiguousarray(np.cos(emb).T.astype(ml_dtypes.bfloat16))  # [128, S]
    sinT = np.ascontiguousarray(np.sin(emb).T.astype(ml_dtypes.bfloat16))

    # S matrix: tmp = S_ @ q gives tmp[p] = -q[p+64] (p<64), q[p-64] (p>=64)
    # matmul computes lhsT.T @ rhs, so pass st = S_^T.
    S_ = np.zeros((128, 128), dtype=np.float32)
    for p in range(64):
        S_[p, p + 64] = -1.0
        S_[p + 64, p] = 1.0
    st = np.ascontiguousarray(S_.T.astype(ml_dtypes.bfloat16))

    ones = np.ones((128, 1), dtype=ml_dtypes.bfloat16)

    in_maps = []
    for c in range(NCORES):
        j0 = c * CW
        bq_c = np.asarray(bq[j0 : j0 + CW], dtype=np.float32)
        bk_c = np.asarray(bk[j0 : j0 + CW], dtype=np.float32)
        bv_c = np.asarray(bv[j0 : j0 + CW], dtype=np.float32)
        # ACT computes in*scale + bias, so pre-scale the q bias columns
        qs = 1.0 / math.sqrt(HD)
        bqk = np.stack(
            [bq_c[:HD] * qs, bq_c[HD:] * qs, bk_c[:HD], bk_c[HD:]], axis=1
        ).astype(np.float32)  # [128, 4]
        in_maps.append(
            {
                "xt": XT,
                "wq": np.ascontiguousarray(
                    np.asarray(Wq[:, j0 : j0 + CW], dtype=np.float32).astype(in_np)
                ),
                "wk": np.ascontiguousarray(
                    np.asarray(Wk[:, j0 : j0 + CW], dtype=np.float32).astype(in_np)
                ),
                "wv": np.ascontiguousarray(
                    np.asarray(Wv[:, j0 : j0 + CW], dtype=np.float32).astype(in_np)
                ),
                "wo": np.ascontiguousarray(np.asarray(Wo[j0 : j0 + CW, :], dtype=np.float32).astype(ml_dtypes.bfloat16)),
                "bqk": np.ascontiguousarray(bqk),
                "bvb": np.ascontiguousarray(
                    np.tile(bv_c[None, :], (128, 1)).astype(ml_dtypes.bfloat16)
                ),
                "cosT": cosT,
                "sinT": sinT,
                "st": st,
                "ones": ones,
            }
        )
    return in_maps


def kernel(hidden_states, Wq, bq, Wk, bk, Wv, bv, Wo):
    from concourse import bass_utils

    nc = _build(repeat=1)
    in_maps = _host_inputs(hidden_states, Wq, bq, Wk, bk, Wv, bv, Wo)
    res = bass_utils.run_bass_kernel_spmd(nc, in_maps, core_ids=list(range(NCORES)))
    acc = res.results[0]["out"].astype(np.float32)
    for c in range(1, NCORES):
        acc = acc + res.results[c]["out"]
    return acc.reshape(B, S, HIDDEN)

